# revision 4
# baseline (speedup 1.0000x reference)
"""Trainium2 Bass kernel for BaseLIDIA weighted overlap-add (fold) network.

Math (derived from the reference):
  out[t,ch,y,x] = 0.5 * img[t,ch,y,x] / cnt[t,y,x] + mean(noisy[t,ch])
  img[ch,y,x]   = sum_{i,j in 0..4} deno[t, (y+4-i)*536 + (x+4-j), ch*25+i*5+j]
                                    * w[t, (y+4-i)*536 + (x+4-j)]
  cnt[y,x]      = sum_{i,j in 0..4} w[t, (y+4-i)*536 + (x+4-j)]
(`inds` is unused by the reference; the pre/post scaling collapses so that the
only use of `noisy` is its raw per-channel mean.)

Sharding: 8 cores = 2 frames x 4 row-bands of 133 output rows. Each core gets
patch rows [133b, 133b+137) (4-row halo) of its frame.

Per-core on-device algorithm (patch columns q on SBUF partitions, host layout
[q, d, r] with r padded to 138 so every engine AP is unit-stride innermost):
  - load deno band tile [q<=128, d=75, r=138] bf16 in ONE DMA per x-block
    (20.7KB/partition descriptors stream at HBM line rate), w tile [q, 5*138]
  - wd = deno * w in ONE DVE tensor_tensor (w broadcast over d as the OUTER
    free dim; inner dim unit-stride -> DVE 2x_1P packed mode)
  - img[x, ch, y] = PSUM accumulation of 25 shifted matmuls (one per fold tap
    (i,j)): stationary 0/1 shift matrix (padded to 128 cols so FWL engages)
    handles x+4-j, the rhs AP offset (4-i) handles y+4-i.
  - cnt via 5 matmuls with a single banded 2.0-matrix lhsT (the j-sum folded
    into the stationary operand; the i-shift via rhs AP offset on w); folds
    the final *0.5 into 1/(2 cnt).  No DVE work at all for cnt.
  - rcnt = reciprocal_approx_fast(2 cnt) on DVE; ACT evacuates img PSUM;
    GpSimd does o = img*rcnt + mean; ACT DMAs out as [x, (ch,y)] bf16 — the
    host transposes to [ch, y, x] during assembly.
"""

import ml_dtypes
import numpy as np

import concourse.bass as bass
import concourse.mybir as mybir
import concourse.tile as tile
from concourse import bacc
from concourse.bass_utils import run_bass_kernel_spmd

F32 = mybir.dt.float32
BF16 = mybir.dt.bfloat16
AX = mybir.AxisListType
ALU = mybir.AluOpType
ACTF = mybir.ActivationFunctionType

PS = 5
PH = PW = 536
H = W = 532
PD = 75
NBAND = 4
BAND_Y = 133          # output rows per band
BAND_R = 137          # patch rows per band (halo of PS-1)
RP = 138              # padded patch-row pitch (even -> keeps bf16 2x packing)
NPIX_CH = H * W       # 283024, per-channel pixel count
FD = 3 * BAND_Y       # 399 free elements of the img/out tiles

# x-blocks: (x0, nx, nq)  with q-range [x0, x0 + nq)
XBLKS = [(0, 124, 128), (124, 124, 128), (248, 124, 128), (372, 124, 128),
         (496, 36, 40)]


def _ap_p(base: bass.AP, npart: int, extra_off: int, dims):
    """Custom strided view of a tile: partition dim of `base` overridden to
    `npart`, free dims replaced."""
    part = [[base.ap[0][0], npart]]
    return bass.AP(base.tensor, base.offset + extra_off, part + [list(d) for d in dims])


def build_program(reps: int = 1, ablate: str = ""):
    """Build (and compile) the single-core Bass program. SPMD: all 8 cores run
    it on their own band slice. Returns the Bacc object."""
    nc = bacc.Bacc("TRN2", target_bir_lowering=False, debug=False,
                   enable_asserts=False, num_devices=8)

    deno_d = nc.dram_tensor("deno", [PW, PD, RP], BF16, kind="ExternalInput")
    wt_d = nc.dram_tensor("wt", [128, len(XBLKS) * RP], BF16,
                          kind="ExternalInput")
    noisy_d = nc.dram_tensor("noisy", [3, H, W], BF16, kind="ExternalInput")
    out_d = nc.dram_tensor("out", [W, FD], BF16, kind="ExternalOutput")

    with tile.TileContext(nc) as tc:
        with (
            tc.tile_pool(name="const", bufs=1) as const_p,
            tc.tile_pool(name="deno", bufs=4) as deno_p,
            tc.tile_pool(name="wq", bufs=2) as wq_p,
            tc.tile_pool(name="small", bufs=2) as small_p,
            tc.tile_pool(name="o1", bufs=2) as o1_p,
            tc.tile_pool(name="stage", bufs=3) as stage_p,
            tc.tile_pool(name="noisy", bufs=1) as noisy_p,
            tc.tile_pool(name="psI", bufs=3, space=bass.MemorySpace.PSUM) as psI,
            tc.tile_pool(name="psC", bufs=2, space=bass.MemorySpace.PSUM) as psC,
            tc.tile_pool(name="psW", bufs=1, space=bass.MemorySpace.PSUM) as psW,
        ):
            # ---- constants ----
            # shift identities, padded to 128 cols so FWL engages:
            # shifts[j][q, m] = 1.0 iff q == m + 4 - j
            def mkshift(tag, j, v):
                sh = const_p.tile([128, 128], BF16, tag=tag)
                nc.gpsimd.memset(sh[:], 0.0)
                nc.gpsimd.affine_select(
                    out=sh[:], in_=sh[:], compare_op=ALU.not_equal, fill=v,
                    base=j - 4, pattern=[[-1, 128]], channel_multiplier=1)
                return sh
            shifts = [mkshift(f"shift{j}", j, 1.0) for j in range(PS)]
            # banded cnt matrix: band2[q, m] = 2.0 iff 0 <= q - m <= 4
            # (sum over j of the 5 shift matrices, scaled by 2)
            band2 = const_p.tile([128, 128], BF16, tag="band2")
            nc.gpsimd.memset(band2[:], 0.0)
            for j in range(PS):
                nc.gpsimd.affine_select(
                    out=band2[:], in_=band2[:], compare_op=ALU.not_equal,
                    fill=2.0, base=j - 4, pattern=[[-1, 128]],
                    channel_multiplier=1)

            ones76 = const_p.tile([76, 1], BF16, tag="ones76")
            nc.gpsimd.memset(ones76[:], 1.0)
            onesrow = const_p.tile([1, 128], F32, tag="onesrow")
            nc.gpsimd.memset(onesrow[:], 1.0 / NPIX_CH)

            # ---- per-channel means of raw noisy ----
            sums = const_p.tile([1, 3], F32, tag="sums")
            for ch in range(3):
                npix = noisy_p.tile([76, 3724], BF16, tag="noisy")
                nc.sync.dma_start(
                    out=npix[:],
                    in_=bass.AP(noisy_d, ch * NPIX_CH, [[3724, 76], [1, 3724]]))
                msum = psW.tile([1, 512], F32, tag="psw")
                nchunk = (3724 + 511) // 512
                for ci in range(nchunk):
                    c0 = ci * 512
                    n = min(512, 3724 - c0)
                    nc.tensor.matmul(
                        out=msum[0:1, 0:n],
                        lhsT=ones76[:],
                        rhs=npix[:, c0:c0 + n],
                        start=(ci == 0), stop=(ci == nchunk - 1))
                nc.vector.tensor_reduce(
                    out=sums[0:1, ch:ch + 1], in_=msum[0:1, 0:512],
                    axis=AX.X, op=ALU.add)
            mrep_ps = psW.tile([128, 3], F32, tag="psw")
            nc.tensor.matmul(out=mrep_ps[:], lhsT=onesrow[:],
                             rhs=sums[:], start=True, stop=True)
            mean_rep = const_p.tile([128, 3], F32, tag="mean_rep")
            nc.scalar.copy(mean_rep[:], mrep_ps[:])

            # ---- main loop ----
            # reps>1 wraps the body in a For_i hardware loop (for timing runs)
            UNROLL = 2
            import contextlib
            loop_cm = (tc.For_i(0, (reps + UNROLL - 1) // UNROLL, 1,
                                staggered_reset=True)
                       if reps > 1 else contextlib.nullcontext())
            n_passes = UNROLL if reps > 1 else 1
            if "nomm" in ablate and "nofin" not in ablate:
                ablate = ablate + " nofin"
            with loop_cm:
              # finals are software-pipelined one block behind the front-end
              # so the PE-consuming ops never stall their engine queues
              # waiting on this block's matmuls.
              pend = []
              rc_pend = []
              for _pass in range(n_passes):
                # wq rides the ACT HWDGE ring so the SP ring streams slabs
                # without interruption.
                wq = wq_p.tile([128, len(XBLKS) * RP], BF16, tag="wq")
                nc.scalar.dma_start(out=wq[:], in_=wt_d[:, :])

                # cnt for ALL blocks (depends only on wq): blocks 0-2 in one
                # PSUM tile, 3-4 in another.  Banded lhsT folds the j-sum;
                # the i-shift is the rhs AP offset; the block dim rides the
                # rhs free AP (the band matrix is block-local in q).  Emitted
                # as a closure so the matmuls land in the PE FIFO after block
                # 0's img matmuls (never delaying them), while the per-block
                # reciprocals trail one block behind in the DVE FIFO.
                cntA = psC.tile([128, 3 * BAND_Y], F32, tag="cntA")
                cntB = psC.tile([128, 2 * BAND_Y], F32, tag="cntB")
                rcA = small_p.tile([124, 3 * BAND_Y], F32, tag="rcA")
                rcB = small_p.tile([124, 2 * BAND_Y], F32, tag="rcB")

                def cnt_mms(cntA=cntA, cntB=cntB):
                    for i in range(PS):
                        nc.tensor.matmul(
                            out=cntA[:, :],
                            lhsT=band2[:, :],
                            rhs=_ap_p(wq[:], 128, (4 - i),
                                      [[RP, 3], [1, BAND_Y]]),
                            start=(i == 0), stop=(i == PS - 1))
                    for i in range(PS):
                        nc.tensor.matmul(
                            out=cntB[:, :],
                            lhsT=band2[:, :],
                            rhs=_ap_p(wq[:], 128, 3 * RP + (4 - i),
                                      [[RP, 2], [1, BAND_Y]]),
                            start=(i == 0), stop=(i == PS - 1))

                def mk_rc(b, cntA=cntA, cntB=cntB, rcA=rcA, rcB=rcB):
                    # per-block reciprocal slice (block 4 only has 36 cols;
                    # the rest of its cnt slice is 0 -> skip, 1/0 is inf)
                    src, dst, off = ((cntA, rcA, b) if b < 3 else
                                     (cntB, rcB, b - 3))
                    np_ = 124 if b < 4 else 36
                    def rc():
                        nc.vector.reciprocal_approx_fast(
                            dst[0:np_, off * BAND_Y:(off + 1) * BAND_Y],
                            src[0:np_, off * BAND_Y:(off + 1) * BAND_Y])
                    return rc

                for b, (x0, nx, nq) in enumerate(XBLKS):
                    dt = deno_p.tile([128, PD * RP], BF16, tag="deno")
                    # whole [q, d, r] slab in one DMA: 20.7KB contiguous per
                    # partition streams at HBM line rate.
                    if "nodma" not in ablate:
                        nc.sync.dma_start(
                            out=dt[0:nq, :],
                            in_=bass.AP(deno_d, x0 * PD * RP,
                                        [[PD * RP, nq], [1, PD * RP]]))
                    img = None
                    if "nomm" not in ablate:
                        img = psI.tile([128, FD], F32, tag="img")
                    # wd = deno * w in 5 j-chunks (d = j mod 5); each chunk
                    # immediately feeds its 5 img taps so the PE wakes every
                    # ~1.2us and the HAM clock gate stays at full rate.
                    for j in range(PS):
                        if "nott" not in ablate:
                            nc.vector.tensor_tensor(
                                out=_ap_p(dt[:], nq, j * RP,
                                          [[5 * RP, 15], [1, RP]]),
                                in0=_ap_p(dt[:], nq, j * RP,
                                          [[5 * RP, 15], [1, RP]]),
                                in1=_ap_p(wq[:], nq, b * RP,
                                          [[0, 15], [1, RP]]),
                                op=ALU.mult)
                        # img[x, (ch,y)] accumulates the 5 taps of this j:
                        # tap (i,j): rhs = wd[q, d=ch*25+i*5+j, r=y+4-i]
                        if "nomm" not in ablate:
                            for i in range(PS):
                                nc.tensor.matmul(
                                    out=img[:, :],
                                    lhsT=shifts[j][0:nq, :],
                                    rhs=_ap_p(dt[:], nq,
                                              (i * PS + j) * RP + (4 - i),
                                              [[25 * RP, 3], [1, BAND_Y]]),
                                    start=(j == 0 and i == 0),
                                    stop=(j == PS - 1 and i == PS - 1))
                    if b == 0 and "nomm" not in ablate:
                        cnt_mms()

                    # finals (deferred 1 block): ACT evacuates img PSUM,
                    # GpSimd elementwise, ACT out DMA.  rcnt comes from the
                    # batched per-pass tiles.
                    def finals(b=b, x0=x0, nx=nx, img=img, rcA=rcA, rcB=rcB):
                        st = stage_p.tile([124, FD], BF16, tag="st")
                        if "nofin" not in ablate:
                            rc_t, off = (rcA, b) if b < 3 else (rcB, b - 3)
                            o1 = o1_p.tile([124, FD], BF16, tag="o1")
                            nc.scalar.copy(o1[0:nx, :], img[0:nx, :])
                            nc.gpsimd.tensor_tensor(
                                out=st[0:nx, :],
                                in0=o1[0:nx, :],
                                in1=_ap_p(rc_t[:], nx, off * BAND_Y,
                                          [[0, 3], [1, BAND_Y]]),
                                op=ALU.mult)
                            nc.gpsimd.tensor_tensor(
                                out=st[0:nx, :],
                                in0=st[0:nx, :],
                                in1=_ap_p(mean_rep[:], nx, 0,
                                          [[1, 3], [0, BAND_Y]]),
                                op=ALU.add)
                        else:
                            nc.gpsimd.memset(st[:], 0.0)
                        if "noout" not in ablate:
                            nc.scalar.dma_start(
                                out=bass.AP(out_d, x0 * FD,
                                            [[FD, nx], [1, FD]]),
                                in_=st[0:nx, :])
                    if "nomm" not in ablate and "nofin" not in ablate:
                        rc_pend.append(mk_rc(b))
                        if len(rc_pend) > 1:
                            rc_pend.pop(0)()
                    pend.append(finals)
                    if len(pend) > 1:
                        pend.pop(0)()
              for rc in rc_pend:
                  rc()
              rc_pend.clear()
              for f in pend:
                  f()
              pend.clear()

    nc.compile()
    return nc


_CACHE = {}


def _get_program(reps: int = 1, ablate: str = ""):
    key = (reps, ablate)
    if key not in _CACHE:
        _CACHE[key] = build_program(reps, ablate)
    return _CACHE[key]


def make_in_maps(noisy, deno, patch_weights):
    in_maps = []
    bf = ml_dtypes.bfloat16
    for core in range(8):
        t, b = divmod(core, NBAND)
        dband = deno[t].reshape(PH, PW, PD)[133 * b:133 * b + BAND_R]
        dband = dband.transpose(1, 2, 0)          # [q=536, d=75, r=137]
        dpad = np.zeros((PW, PD, RP), dtype=bf)
        dpad[:, :, :BAND_R] = dband.astype(bf)
        wband = patch_weights[t, :, 0].reshape(PH, PW)[133 * b:133 * b + BAND_R]
        wband = wband.T                            # [q=536, r=137]
        wtile = np.zeros((128, len(XBLKS) * RP), dtype=bf)
        for blk, (x0, nx, nq) in enumerate(XBLKS):
            wtile[0:nq, blk * RP:blk * RP + BAND_R] = \
                wband[x0:x0 + nq].astype(bf)
        in_maps.append({
            "deno": dpad,
            "wt": wtile,
            "noisy": np.ascontiguousarray(noisy[t]).astype(bf),
        })
    return in_maps


def unpack_out(arr):
    """Device out [532, 399] bf16 -> [3, 133, 532] f32."""
    a = np.asarray(arr).astype(np.float32).reshape(W, 3, BAND_Y)
    return a.transpose(1, 2, 0)


def assemble(results):
    out = np.empty((2, 3, H, W), dtype=np.float32)
    for core in range(8):
        t, b = divmod(core, NBAND)
        out[t, :, 133 * b:133 * b + BAND_Y, :] = unpack_out(results[core]["out"])
    return out


def kernel(noisy, deno, patch_weights, inds=None, pixels_h=None, pixels_w=None,
           patches_h=None, patches_w=None, **_):
    noisy = np.asarray(noisy, dtype=np.float32)
    deno = np.asarray(deno, dtype=np.float32)
    patch_weights = np.asarray(patch_weights, dtype=np.float32)
    nc = _get_program()
    res = run_bass_kernel_spmd(nc, make_in_maps(noisy, deno, patch_weights),
                               core_ids=list(range(8)))
    return assemble(res.results)
